# revision 10
# baseline (speedup 1.0000x reference)
"""Trainium2 Bass kernel for nn_BreakthroughSNN (spiking SSM LM).

Strategy (8 NeuronCores, SPMD single NEFF):
  - Data-parallel SSM: 2048 tokens (B*S) sharded 256/core. Per core, the
    4-layer x 20-step LIF recurrence runs with persistent membrane
    potentials held in PSUM (PE accumulates state/output updates, DVE
    applies the leak*reset in place, ACT computes sign(v-thr), GPSIMD
    emits spikes).
  - All SSM matmuls are fp32r hi/lo pairs (host-split so the device's
    fp32r rounding is exact) -> full fp32-grade precision at 1 cyc/row.
  - Temporal encoding via host-precomputed exact fp32 sigmoid-boundary
    thresholds (no device sigmoid -> bit-exact one-hot vs fp32 ref).
  - Vocab-sharded output projection: time-integrated rates are
    AllGathered (bf16, tiny) so each core computes all 2048 tokens x its
    4000-vocab shard; Wp streamed as bf16 (post-chaos linear op).
"""

import numpy as np
import ml_dtypes
from contextlib import ExitStack

import concourse.bass as bass
import concourse.mybir as mybir
import concourse.tile as tile
from concourse import bacc
from concourse.bass_utils import run_bass_kernel_spmd
from concourse.masks import make_identity

F32 = mybir.dt.float32
F32R = mybir.dt.float32r
BF16 = mybir.dt.bfloat16
I32 = mybir.dt.int32
OP = mybir.AluOpType
ACTF = mybir.ActivationFunctionType

NCORES = 8
TOKPC = 256          # tokens per core
BATCH, SEQ = 4, 512
DM, DS = 512, 128
T, L = 20, 4
VOC = 32000
VSH = VOC // NCORES  # 4000 vocab per core
NV = 500             # vocab cols per proj tile (one PSUM bank; 8 tiles per core)
KC = DM // 128       # 4 feature chunks


def _hilo(x):
    x = np.ascontiguousarray(x, dtype=np.float32)
    u = x.view(np.uint32)
    hi = (u & np.uint32(0xFFFFF000)).view(np.float32).copy()  # keep 11 mantissa bits
    lo = (x - hi).astype(np.float32)
    return hi, lo


def _f2key(x):
    u = int(np.array(x, dtype=np.float32).view(np.uint32))
    return (u ^ 0x80000000) if u < 0x80000000 else (0xFFFFFFFF - u)


def _key2f(k):
    u = (k ^ 0x80000000) if k >= 0x80000000 else (0xFFFFFFFF - k)
    return np.array([u], dtype=np.uint32).view(np.float32)[0]


def _g32(x):
    # replicate reference fp32 pipeline: floor happens on this value
    x = np.float32(x)
    s = np.float32(1.0) / (np.float32(1.0) + np.float32(np.exp(np.float32(-x))))
    return np.float32(s * np.float32(19.0))


def _thresholds():
    """T_k = smallest fp32 x with g32(x) >= k, k=1..19 (g32 monotone)."""
    ts = []
    for k in range(1, 20):
        lo_k = _f2key(np.float32(-30.0))
        hi_k = _f2key(np.float32(30.0))
        assert _g32(_key2f(hi_k)) >= k and _g32(_key2f(lo_k)) < k
        while hi_k - lo_k > 1:
            mid = (lo_k + hi_k) // 2
            if _g32(_key2f(mid)) >= k:
                hi_k = mid
            else:
                lo_k = mid
        ts.append(float(_key2f(hi_k)))
    return ts


def _build_nc():
    nc = bacc.Bacc("TRN2", target_bir_lowering=False, debug=False, num_devices=NCORES)

    ids_d = nc.dram_tensor("ids", [2, 128, 1], I32, kind="ExternalInput")
    emb_d = nc.dram_tensor("emb", [VOC, DM], F32, kind="ExternalInput")
    at_hi_d = nc.dram_tensor("at_hi", [L, 128, 128], F32, kind="ExternalInput")
    at_lo_d = nc.dram_tensor("at_lo", [L, 128, 128], F32, kind="ExternalInput")
    bt_hi_d = nc.dram_tensor("bt_hi", [L, 128, KC, 128], F32, kind="ExternalInput")
    bt_lo_d = nc.dram_tensor("bt_lo", [L, 128, KC, 128], F32, kind="ExternalInput")
    ct_hi_d = nc.dram_tensor("ct_hi", [L, 128, KC, 128], F32, kind="ExternalInput")
    ct_lo_d = nc.dram_tensor("ct_lo", [L, 128, KC, 128], F32, kind="ExternalInput")
    dc_hi_d = nc.dram_tensor("dc_hi", [L, 128, KC], F32, kind="ExternalInput")
    dc_lo_d = nc.dram_tensor("dc_lo", [L, 128, KC], F32, kind="ExternalInput")
    wpt_d = nc.dram_tensor("wpt", [DM, VSH], BF16, kind="ExternalInput")
    bias_d = nc.dram_tensor("bias", [1, VSH], F32, kind="ExternalInput")
    out_d = nc.dram_tensor("out", [TOKPC * NCORES, VSH], F32, kind="ExternalOutput")

    THR = _thresholds()

    with tile.TileContext(nc) as tc, ExitStack() as ctx:
        const = ctx.enter_context(tc.tile_pool(name="const", bufs=1))
        ident = const.tile([128, 128], F32)
        make_identity(nc, ident[:])
        neg2 = const.tile([128, 1], F32)
        nc.vector.memset(neg2[:], -2.0)

        xb_pool = ctx.enter_context(tc.tile_pool(name="xb", bufs=1))
        xb = xb_pool.tile([128, T * KC * 256], F32R)

        # ---------------- encode: gather + transpose + thresholds ----------
        with tc.tile_pool(name="enc", bufs=2) as enc, \
             tc.tile_pool(name="encp", bufs=2, space="PSUM") as encps, \
             tc.tile_pool(name="emb4", bufs=1) as emb4:
            ids_s = enc.tile([128, 2], I32, tag="ids")
            for g in range(2):
                nc.sync.dma_start(ids_s[:, g:g + 1], ids_d[g, :, :])
            EMB = [emb4.tile([128, TOKPC], F32, tag=f"emb{k}", name=f"EMB{k}") for k in range(KC)]
            IDX = [emb4.tile([128, TOKPC], F32, tag=f"idx{k}", name=f"IDX{k}") for k in range(KC)]
            for g in range(2):
                eg = enc.tile([128, DM], F32, tag="eg")
                nc.gpsimd.indirect_dma_start(
                    out=eg[:], out_offset=None,
                    in_=emb_d[:, :],
                    in_offset=bass.IndirectOffsetOnAxis(ap=ids_s[:, g:g + 1], axis=0),
                )
                for k in range(KC):
                    pt = encps.tile([128, 128], F32, tag="pt")
                    nc.tensor.transpose(pt[:], eg[:, k * 128:(k + 1) * 128], ident[:])
                    nc.scalar.copy(EMB[k][:, g * 128:(g + 1) * 128], pt[:])
            engs = [nc.vector, nc.gpsimd]
            for k in range(KC):
                nc.vector.memset(IDX[k][:], 0.0)
                for j, tj in enumerate(THR):
                    nc.vector.scalar_tensor_tensor(IDX[k][:], EMB[k][:], float(tj),
                                                   IDX[k][:], OP.is_ge, OP.add)
            # one-hot spikes into X buffer (values {0,1}, fp32r-exact)
            for t in range(T):
                for k in range(KC):
                    e = engs[(t + k) % 2]
                    e.tensor_scalar(xb[:, (t * KC + k) * 256:(t * KC + k) * 256 + 256],
                                    IDX[k][:], float(t), None, OP.is_equal)

        # ---------------- SSM layers ---------------------------------------
        with tc.tile_pool(name="ssmp", bufs=1, space="PSUM") as ssmps, \
             tc.tile_pool(name="par", bufs=2) as par, \
             tc.tile_pool(name="stg", bufs=2) as stg, \
             tc.tile_pool(name="lif", bufs=3) as lif:
            v1ps = ssmps.tile([128, TOKPC], F32, tag="v1")
            v2ps = [ssmps.tile([128, TOKPC], F32, tag=f"v2_{k}", name=f"v2ps{k}") for k in range(KC)]

            Hprev = None
            for layer in range(L):
                # -- param prep (hi/lo fp32r tiles; host pre-rounded) --
                def load_rounded(dram_ap, shape, tag):
                    st = stg.tile(list(shape), F32, tag="stage")
                    nc.sync.dma_start(st[:], dram_ap)
                    pt_ = par.tile(list(shape), F32R, tag=tag, name=f"par_{tag}")
                    nc.vector.tensor_copy(pt_[:], st[:])
                    return pt_

                ah = load_rounded(at_hi_d[layer, :, :], (128, 128), "ah")
                al = load_rounded(at_lo_d[layer, :, :], (128, 128), "al")
                bh = load_rounded(bt_hi_d[layer, :, :, :], (128, KC, 128), "bh")
                bl = load_rounded(bt_lo_d[layer, :, :, :], (128, KC, 128), "bl")
                ch = load_rounded(ct_hi_d[layer, :, :, :], (128, KC, 128), "ch")
                cl = load_rounded(ct_lo_d[layer, :, :, :], (128, KC, 128), "cl")
                dch = stg.tile([128, KC], F32, tag="dch")
                nc.sync.dma_start(dch[:], dc_hi_d[layer, :, :])
                dcl = stg.tile([128, KC], F32, tag="dcl")
                nc.sync.dma_start(dcl[:], dc_lo_d[layer, :, :])
                ddh, ddl = [], []
                for k in range(KC):
                    dt_ = par.tile([128, 128], F32R, tag=f"ddh{k}", name=f"ddh{k}")
                    nc.gpsimd.tensor_scalar(dt_[:], ident[:], dch[:, k:k + 1], None, OP.mult)
                    ddh.append(dt_)
                    dt_ = par.tile([128, 128], F32R, tag=f"ddl{k}", name=f"ddl{k}")
                    nc.gpsimd.tensor_scalar(dt_[:], ident[:], dcl[:, k:k + 1], None, OP.mult)
                    ddl.append(dt_)

                for t in range(T):
                    xs = [xb[:, (t * KC + k) * 256:(t * KC + k) * 256 + 256]
                          for k in range(KC)]
                    # ---- state update accumulation (v1) ----
                    mm1 = []
                    if t > 0:
                        mm1 += [(ah[:], Hprev[:]), (al[:], Hprev[:])]
                    for k in range(KC):
                        mm1 += [(bh[:, k, :], xs[k]), (bl[:, k, :], xs[k])]
                    for i, (lhsT, rhs) in enumerate(mm1):
                        nc.tensor.matmul(v1ps[:], lhsT, rhs,
                                         start=(t == 0 and i == 0),
                                         stop=(i == len(mm1) - 1),
                                         skip_group_check=True)
                    # ---- LIF1 ----
                    sg1 = lif.tile([128, TOKPC], F32, tag="sg1")
                    nc.scalar.activation(sg1[:], v1ps[:], ACTF.Sign, bias=neg2[:], scale=1.0)
                    m1 = lif.tile([128, TOKPC], F32, tag="m1")
                    nc.vector.tensor_scalar(m1[:], sg1[:], -0.25, 0.25, OP.mult, OP.add)
                    H = lif.tile([128, TOKPC], F32R, tag="H")
                    nc.gpsimd.tensor_scalar(H[:], sg1[:], 0.5, 0.5, OP.mult, OP.add)
                    nc.vector.tensor_tensor(v1ps[:], v1ps[:], m1[:], OP.mult)
                    # ---- output update accumulation (v2, per chunk) ----
                    for k in range(KC):
                        mm2 = [(ch[:, k, :], H[:]), (cl[:, k, :], H[:]),
                               (ddh[k][:], xs[k]), (ddl[k][:], xs[k])]
                        for i, (lhsT, rhs) in enumerate(mm2):
                            nc.tensor.matmul(v2ps[k][:], lhsT, rhs,
                                             start=(t == 0 and i == 0),
                                             stop=(i == len(mm2) - 1),
                                             skip_group_check=True)
                    # ---- LIF2 per chunk; spikes overwrite X[t] in place ----
                    for k in range(KC):
                        sg2 = lif.tile([128, TOKPC], F32, tag=f"sg2_{k}")
                        nc.scalar.activation(sg2[:], v2ps[k][:], ACTF.Sign,
                                             bias=neg2[:], scale=1.0)
                        m2 = lif.tile([128, TOKPC], F32, tag=f"m2_{k}")
                        nc.vector.tensor_scalar(m2[:], sg2[:], -0.25, 0.25, OP.mult, OP.add)
                        nc.gpsimd.tensor_scalar(xs[k], sg2[:], 0.5, 0.5, OP.mult, OP.add)
                        nc.vector.tensor_tensor(v2ps[k][:], v2ps[k][:], m2[:], OP.mult)
                    Hprev = H

        # ---------------- time integration + allgather ---------------------
        with tc.tile_pool(name="ti", bufs=1) as tip, \
             tc.tile_pool(name="agd", bufs=1, space="DRAM") as agd:
            tibf = tip.tile([128, KC * 256], BF16, tag="tibf")
            xv = xb[:].bitcast(F32).rearrange("p (t k c) -> p k c t", t=T, k=KC, c=256)
            for k in range(KC):
                tik = tip.tile([128, 256], F32, tag=f"ti{k}", name=f"tik{k}")
                nc.vector.tensor_reduce(tik[:], xv[:, k, :, :],
                                        mybir.AxisListType.X, OP.add)
                nc.vector.tensor_scalar(tibf[:, k * 256:(k + 1) * 256], tik[:],
                                        1.0 / T, None, OP.mult)
            agi = agd.tile([128, KC * 256], BF16)
            nc.sync.dma_start(agi[:], tibf[:])
            ago = agd.tile([NCORES * 128, KC * 256], BF16)
            nc.gpsimd.collective_compute(
                "AllGather", OP.bypass,
                replica_groups=[list(range(NCORES))],
                ins=[agi[:].opt()], outs=[ago[:].opt()],
            )
            tiall = tip.tile([128, NCORES, KC * 256], BF16, tag="tiall")
            nc.sync.dma_start(
                tiall[:],
                ago[:].rearrange("(n p) x -> p n x", n=NCORES, p=128))

            # ---------------- vocab-sharded projection ---------------------
            with tc.tile_pool(name="prj", bufs=2) as prj, \
                 tc.tile_pool(name="prjp", bufs=2, space="PSUM") as prjps, \
                 tc.tile_pool(name="osb", bufs=3) as osbp:
                for nv in range(VSH // NV):
                    bias_bc = prj.tile([128, NV], F32, tag="bias")
                    bap = bias_d[0:1, nv * NV:(nv + 1) * NV]
                    bsrc = bass.AP(tensor=bap.tensor, offset=bap.offset,
                                   ap=[[0, 128], [1, NV]])
                    nc.sync.dma_start(bias_bc[:], bsrc)
                    wts = []
                    for k in range(KC):
                        wt = prj.tile([128, NV], BF16, tag=f"wt{k}")
                        nc.sync.dma_start(wt[:], wpt_d[k * 128:(k + 1) * 128,
                                                       nv * NV:(nv + 1) * NV])
                        wts.append(wt)
                    for m in range(TOKPC * NCORES // 128):
                        c, half = divmod(m, 2)
                        po = prjps.tile([128, NV], F32, tag="po")
                        for k in range(KC):
                            lh = tiall[:, c, k * 256 + half * 128:
                                       k * 256 + half * 128 + 128]
                            nc.tensor.matmul(po[:], lh, wts[k][:],
                                             start=(k == 0), stop=(k == KC - 1),
                                             skip_group_check=True)
                        osb = osbp.tile([128, NV], F32, tag="osb")
                        nc.vector.tensor_tensor(osb[:], po[:], bias_bc[:], OP.add)
                        nc.sync.dma_start(out_d[m * 128:(m + 1) * 128,
                                                nv * NV:(nv + 1) * NV], osb[:])

    nc.compile()
    return nc


_NC_CACHE = {}
_last_in_maps = None


def _get_nc():
    if "nc" not in _NC_CACHE:
        _NC_CACHE["nc"] = _build_nc()
    return _NC_CACHE["nc"]


def kernel(input_ids, emb_table, A, B, C, D, Wp, bp):
    input_ids = np.asarray(input_ids)
    emb_table = np.ascontiguousarray(np.asarray(emb_table), dtype=np.float32)
    A = np.asarray(A, dtype=np.float32)
    B = np.asarray(B, dtype=np.float32)
    C = np.asarray(C, dtype=np.float32)
    D = np.asarray(D, dtype=np.float32)
    Wp = np.asarray(Wp, dtype=np.float32)
    bp = np.asarray(bp, dtype=np.float32)

    ids_flat = input_ids.reshape(-1).astype(np.int32)          # (2048,)

    at = np.ascontiguousarray(A.transpose(0, 2, 1))            # (L,128,128)
    at_hi, at_lo = _hilo(at)
    bt = np.ascontiguousarray(
        B.transpose(2, 0, 1).reshape(KC, 128, L, DS).transpose(2, 1, 0, 3))
    # bt[l,p,k,m] = B[l, m, k*128+p]
    bt_hi, bt_lo = _hilo(bt)
    ct = np.ascontiguousarray(C.transpose(0, 2, 1).reshape(L, 128, KC, 128))
    # ct[l,p,mc,m] = C[l, mc*128+m, p]
    ct_hi, ct_lo = _hilo(ct)
    dc = np.ascontiguousarray(D.reshape(L, KC, 128).transpose(0, 2, 1))  # (L,128,KC)
    dc_hi, dc_lo = _hilo(dc)

    wpt = np.ascontiguousarray(Wp.T)                           # (512, 32000) f32
    wpt_bf = wpt.astype(ml_dtypes.bfloat16)

    nc = _get_nc()
    in_maps = []
    for c in range(NCORES):
        ids_c = ids_flat[c * TOKPC:(c + 1) * TOKPC].reshape(2, 128, 1)
        in_maps.append({
            "ids": np.ascontiguousarray(ids_c),
            "emb": emb_table,
            "at_hi": at_hi, "at_lo": at_lo,
            "bt_hi": bt_hi, "bt_lo": bt_lo,
            "ct_hi": ct_hi, "ct_lo": ct_lo,
            "dc_hi": dc_hi, "dc_lo": dc_lo,
            "wpt": np.ascontiguousarray(wpt_bf[:, c * VSH:(c + 1) * VSH]),
            "bias": np.ascontiguousarray(bp[c * VSH:(c + 1) * VSH]).reshape(1, VSH),
        })

    global _last_in_maps
    _last_in_maps = in_maps
    res = run_bass_kernel_spmd(nc, in_maps, core_ids=list(range(NCORES)))
    outs = [res.results[c]["out"] for c in range(NCORES)]
    full = np.concatenate(outs, axis=1)                        # (2048, 32000)
    return full.reshape(BATCH, SEQ, VOC).astype(np.float32)


# revision 15
# speedup vs baseline: 1.4527x; 1.4527x over previous
"""Trainium2 Bass kernel for nn_BreakthroughSNN (spiking SSM LM).

Strategy (8 NeuronCores, SPMD single NEFF):
  - Data-parallel SSM: 2048 tokens (B*S) sharded 256/core. Per core, the
    4-layer x 20-step LIF recurrence runs with persistent membrane
    potentials held in PSUM (PE accumulates state/output updates, DVE
    applies the leak*reset in place, ACT computes sign(v-thr), GPSIMD
    emits spikes).
  - All SSM matmuls are fp32r hi/lo pairs (host-split so the device's
    fp32r rounding is exact) -> full fp32-grade precision at 1 cyc/row.
  - Temporal encoding via host-precomputed exact fp32 sigmoid-boundary
    thresholds (no device sigmoid -> bit-exact one-hot vs fp32 ref).
  - Vocab-sharded output projection: time-integrated rates are
    AllGathered (bf16, tiny) so each core computes all 2048 tokens x its
    4000-vocab shard; Wp streamed as bf16 (post-chaos linear op).
"""

import numpy as np
import ml_dtypes
from contextlib import ExitStack

import concourse.bass as bass
import concourse.mybir as mybir
import concourse.tile as tile
from concourse import bacc
from concourse.bass_utils import run_bass_kernel_spmd
from concourse.masks import make_identity

F32 = mybir.dt.float32
F32R = mybir.dt.float32r
BF16 = mybir.dt.bfloat16
I32 = mybir.dt.int32
OP = mybir.AluOpType
ACTF = mybir.ActivationFunctionType

NCORES = 8
TOKPC = 256          # tokens per core
BATCH, SEQ = 4, 512
DM, DS = 512, 128
T, L = 20, 4
VOC = 32000
VSH = VOC // NCORES  # 4000 vocab per core
NV = 500             # vocab cols per proj tile (one PSUM bank; 8 tiles per core)
KC = DM // 128       # 4 feature chunks


def _hilo(x):
    x = np.ascontiguousarray(x, dtype=np.float32)
    u = x.view(np.uint32)
    hi = (u & np.uint32(0xFFFFF000)).view(np.float32).copy()  # keep 11 mantissa bits
    lo = (x - hi).astype(np.float32)
    return hi, lo


def _f2key(x):
    u = int(np.array(x, dtype=np.float32).view(np.uint32))
    return (u ^ 0x80000000) if u < 0x80000000 else (0xFFFFFFFF - u)


def _key2f(k):
    u = (k ^ 0x80000000) if k >= 0x80000000 else (0xFFFFFFFF - k)
    return np.array([u], dtype=np.uint32).view(np.float32)[0]


def _g32(x):
    # replicate reference fp32 pipeline: floor happens on this value
    x = np.float32(x)
    s = np.float32(1.0) / (np.float32(1.0) + np.float32(np.exp(np.float32(-x))))
    return np.float32(s * np.float32(19.0))


def _thresholds():
    """T_k = smallest fp32 x with g32(x) >= k, k=1..19 (g32 monotone)."""
    ts = []
    for k in range(1, 20):
        lo_k = _f2key(np.float32(-30.0))
        hi_k = _f2key(np.float32(30.0))
        assert _g32(_key2f(hi_k)) >= k and _g32(_key2f(lo_k)) < k
        while hi_k - lo_k > 1:
            mid = (lo_k + hi_k) // 2
            if _g32(_key2f(mid)) >= k:
                hi_k = mid
            else:
                lo_k = mid
        ts.append(float(_key2f(hi_k)))
    return ts


def _build_nc():
    nc = bacc.Bacc("TRN2", target_bir_lowering=False, debug=False, num_devices=NCORES)

    ids_d = nc.dram_tensor("ids", [2, 128, 1], I32, kind="ExternalInput")
    emb_d = nc.dram_tensor("emb", [VOC, DM], F32, kind="ExternalInput")
    at_hi_d = nc.dram_tensor("at_hi", [L, 128, 128], F32, kind="ExternalInput")
    at_lo_d = nc.dram_tensor("at_lo", [L, 128, 128], F32, kind="ExternalInput")
    bt_hi_d = nc.dram_tensor("bt_hi", [L, 128, KC, 128], F32, kind="ExternalInput")
    bt_lo_d = nc.dram_tensor("bt_lo", [L, 128, KC, 128], F32, kind="ExternalInput")
    ct_hi_d = nc.dram_tensor("ct_hi", [L, 128, KC, 128], F32, kind="ExternalInput")
    ct_lo_d = nc.dram_tensor("ct_lo", [L, 128, KC, 128], F32, kind="ExternalInput")
    dc_hi_d = nc.dram_tensor("dc_hi", [L, 128, KC], F32, kind="ExternalInput")
    dc_lo_d = nc.dram_tensor("dc_lo", [L, 128, KC], F32, kind="ExternalInput")
    wpt_d = nc.dram_tensor("wpt", [DM, VSH], BF16, kind="ExternalInput")
    bias_d = nc.dram_tensor("bias", [1, VSH], F32, kind="ExternalInput")
    out_d = nc.dram_tensor("out", [TOKPC * NCORES, VSH], F32, kind="ExternalOutput")

    THR = _thresholds()

    with tile.TileContext(nc) as tc, ExitStack() as ctx:
        const = ctx.enter_context(tc.tile_pool(name="const", bufs=1))
        ident = const.tile([128, 128], F32)
        make_identity(nc, ident[:])
        neg2 = const.tile([128, 1], F32)
        nc.vector.memset(neg2[:], -2.0)

        xb_pool = ctx.enter_context(tc.tile_pool(name="xb", bufs=1))
        xb = xb_pool.tile([128, T * KC * 256], F32R)

        # ---------------- encode: gather + transpose + thresholds ----------
        with tc.tile_pool(name="enc", bufs=2) as enc, \
             tc.tile_pool(name="encp", bufs=2, space="PSUM") as encps, \
             tc.tile_pool(name="emb4", bufs=1) as emb4:
            ids_s = enc.tile([128, 2], I32, tag="ids")
            for g in range(2):
                nc.sync.dma_start(ids_s[:, g:g + 1], ids_d[g, :, :])
            EMB = [emb4.tile([128, TOKPC], F32, tag=f"emb{k}", name=f"EMB{k}") for k in range(KC)]
            IDX = [emb4.tile([128, TOKPC], F32, tag=f"idx{k}", name=f"IDX{k}") for k in range(KC)]
            for g in range(2):
                eg = enc.tile([128, DM], F32, tag="eg")
                nc.gpsimd.indirect_dma_start(
                    out=eg[:], out_offset=None,
                    in_=emb_d[:, :],
                    in_offset=bass.IndirectOffsetOnAxis(ap=ids_s[:, g:g + 1], axis=0),
                )
                for k in range(KC):
                    pt = encps.tile([128, 128], F32, tag="pt")
                    nc.tensor.transpose(pt[:], eg[:, k * 128:(k + 1) * 128], ident[:])
                    nc.scalar.copy(EMB[k][:, g * 128:(g + 1) * 128], pt[:])
            for k in range(KC):
                nc.vector.memset(IDX[k][:], 0.0)
                for j, tj in enumerate(THR):
                    nc.vector.scalar_tensor_tensor(IDX[k][:], EMB[k][:], float(tj),
                                                   IDX[k][:], OP.is_ge, OP.add)
            # one-hot spikes into X buffer (values {0,1}, fp32r-exact)
            for t in range(T):
                for k in range(KC):
                    nc.vector.tensor_scalar(
                        xb[:, (t * KC + k) * 256:(t * KC + k) * 256 + 256],
                        IDX[k][:], float(t), None, OP.is_equal)

        # ---------------- SSM layers ---------------------------------------
        with tc.tile_pool(name="ssmp", bufs=1, space="PSUM") as ssmps, \
             tc.tile_pool(name="par", bufs=2) as par, \
             tc.tile_pool(name="stg", bufs=2) as stg, \
             tc.tile_pool(name="lif", bufs=3) as lif:
            v1ps = ssmps.tile([128, TOKPC], F32, tag="v1")
            v2ps = [ssmps.tile([128, TOKPC], F32, tag=f"v2_{k}", name=f"v2ps{k}") for k in range(KC)]

            Hprev = None
            for layer in range(L):
                # -- param prep (hi/lo fp32r tiles; host pre-rounded) --
                def load_rounded(dram_ap, shape, tag):
                    st = stg.tile(list(shape), F32, tag="stage")
                    nc.sync.dma_start(st[:], dram_ap)
                    pt_ = par.tile(list(shape), F32R, tag=tag, name=f"par_{tag}")
                    nc.vector.tensor_copy(pt_[:], st[:])
                    return pt_

                ah = load_rounded(at_hi_d[layer, :, :], (128, 128), "ah")
                al = load_rounded(at_lo_d[layer, :, :], (128, 128), "al")
                bh = load_rounded(bt_hi_d[layer, :, :, :], (128, KC, 128), "bh")
                bl = load_rounded(bt_lo_d[layer, :, :, :], (128, KC, 128), "bl")
                ch = load_rounded(ct_hi_d[layer, :, :, :], (128, KC, 128), "ch")
                cl = load_rounded(ct_lo_d[layer, :, :, :], (128, KC, 128), "cl")
                dch = stg.tile([128, KC], F32, tag="dch")
                nc.sync.dma_start(dch[:], dc_hi_d[layer, :, :])
                dcl = stg.tile([128, KC], F32, tag="dcl")
                nc.sync.dma_start(dcl[:], dc_lo_d[layer, :, :])
                ddh, ddl = [], []
                for k in range(KC):
                    dt_ = par.tile([128, 128], F32R, tag=f"ddh{k}", name=f"ddh{k}")
                    nc.vector.tensor_scalar(dt_[:], ident[:], dch[:, k:k + 1], None, OP.mult)
                    ddh.append(dt_)
                    dt_ = par.tile([128, 128], F32R, tag=f"ddl{k}", name=f"ddl{k}")
                    nc.vector.tensor_scalar(dt_[:], ident[:], dcl[:, k:k + 1], None, OP.mult)
                    ddl.append(dt_)

                def emit_mm2_lif2(t, H_t, xs_t):
                    # output update accumulation (v2, per chunk) + LIF2
                    for k in range(KC):
                        mm2 = [(ch[:, k, :], H_t[:]), (cl[:, k, :], H_t[:]),
                               (ddh[k][:], xs_t[k]), (ddl[k][:], xs_t[k])]
                        for i, (lhsT, rhs) in enumerate(mm2):
                            nc.tensor.matmul(v2ps[k][:], lhsT, rhs,
                                             start=(t == 0 and i == 0),
                                             stop=(i == len(mm2) - 1),
                                             skip_group_check=True)
                    for k in range(KC):
                        sg2 = lif.tile([128, TOKPC], F32, tag=f"sg2_{k}",
                                       name=f"sg2_{k}")
                        nc.scalar.activation(sg2[:], v2ps[k][:], ACTF.Sign,
                                             bias=neg2[:], scale=1.0)
                        m2 = lif.tile([128, TOKPC], F32, tag=f"m2_{k}",
                                      name=f"m2_{k}")
                        me = nc.gpsimd if k < 2 else nc.vector
                        me.tensor_scalar(m2[:], sg2[:], -0.25, 0.25, OP.mult, OP.add)
                        nc.gpsimd.tensor_scalar(xs_t[k], sg2[:], 0.5, 0.5,
                                                OP.mult, OP.add)
                        nc.vector.tensor_tensor(v2ps[k][:], v2ps[k][:], m2[:], OP.mult)

                prev = None  # (t, H, xs) pending MM2+LIF2 (1-step software skew)
                for t in range(T):
                    xs = [xb[:, (t * KC + k) * 256:(t * KC + k) * 256 + 256]
                          for k in range(KC)]
                    # ---- state update accumulation (v1) ----
                    mm1 = []
                    if t > 0:
                        mm1 += [(ah[:], Hprev[:]), (al[:], Hprev[:])]
                    for k in range(KC):
                        mm1 += [(bh[:, k, :], xs[k]), (bl[:, k, :], xs[k])]
                    for i, (lhsT, rhs) in enumerate(mm1):
                        nc.tensor.matmul(v1ps[:], lhsT, rhs,
                                         start=(t == 0 and i == 0),
                                         stop=(i == len(mm1) - 1),
                                         skip_group_check=True)
                    # ---- LIF1 (m1 straight from PSUM; spike via GPSIMD) ----
                    m1 = lif.tile([128, TOKPC], F32, tag="m1")
                    nc.vector.tensor_scalar(m1[:], v1ps[:], 2.0, 0.5, OP.is_lt, OP.mult)
                    H = lif.tile([128, TOKPC], F32R, tag="H", bufs=3)
                    nc.gpsimd.tensor_scalar(H[:], m1[:], -2.0, 1.0, OP.mult, OP.add)
                    nc.vector.tensor_tensor(v1ps[:], v1ps[:], m1[:], OP.mult)
                    # ---- previous step's output-side work (keeps PE fed) ----
                    if prev is not None:
                        emit_mm2_lif2(*prev)
                    prev = (t, H, xs)
                    Hprev = H
                emit_mm2_lif2(*prev)

        # ---------------- time integration + allgather ---------------------
        with tc.tile_pool(name="ti", bufs=1) as tip, \
             tc.tile_pool(name="agd", bufs=1, space="DRAM") as agd:
            tibf = tip.tile([128, KC * 256], BF16, tag="tibf")
            xv = xb[:].bitcast(F32).rearrange("p (t k c) -> p k c t", t=T, k=KC, c=256)
            for k in range(KC):
                tik = tip.tile([128, 256], F32, tag=f"ti{k}", name=f"tik{k}")
                nc.vector.tensor_reduce(tik[:], xv[:, k, :, :],
                                        mybir.AxisListType.X, OP.add)
                nc.vector.tensor_scalar(tibf[:, k * 256:(k + 1) * 256], tik[:],
                                        1.0 / T, None, OP.mult)
            agi = agd.tile([128, KC * 256], BF16)
            nc.sync.dma_start(agi[:], tibf[:])
            ago = agd.tile([NCORES * 128, KC * 256], BF16)
            nc.gpsimd.collective_compute(
                "AllGather", OP.bypass,
                replica_groups=[list(range(NCORES))],
                ins=[agi[:].opt()], outs=[ago[:].opt()],
            )
            tiall = tip.tile([128, NCORES, KC * 256], BF16, tag="tiall")
            nc.sync.dma_start(
                tiall[:],
                ago[:].rearrange("(n p) x -> p n x", n=NCORES, p=128))

            # ---------------- vocab-sharded projection ---------------------
            with tc.tile_pool(name="prj", bufs=2) as prj, \
                 tc.tile_pool(name="prjc", bufs=1) as prjc, \
                 tc.tile_pool(name="prjp", bufs=2, space="PSUM") as prjps, \
                 tc.tile_pool(name="osb", bufs=3) as osbp:
                # bias as rank-1 fp32r accumulation: ones(1,128).T @ bias(1,NV)
                ones1f = prjc.tile([1, 128], F32, tag="ones1f")
                nc.vector.memset(ones1f[:], 1.0)
                ones1 = prjc.tile([1, 128], F32R, tag="ones1")
                nc.vector.tensor_copy(ones1[:], ones1f[:])
                bias_f = prjc.tile([1, VSH], F32, tag="bias_f")
                nc.sync.dma_start(bias_f[:], bias_d[:, :])
                bias_r = prjc.tile([1, VSH], F32R, tag="bias_r")
                nc.vector.tensor_copy(bias_r[:], bias_f[:])

                mchunks = TOKPC * NCORES // 128
                for nv in range(VSH // NV):
                    wts = []
                    for k in range(KC):
                        wt = prj.tile([128, NV], BF16, tag=f"wt{k}", name=f"wt{k}")
                        nc.sync.dma_start(wt[:], wpt_d[k * 128:(k + 1) * 128,
                                                       nv * NV:(nv + 1) * NV])
                        wts.append(wt)
                    for m in range(mchunks):
                        c, half = divmod(m, 2)
                        po = prjps.tile([128, NV], F32, tag="po")
                        nc.tensor.matmul(po[:], ones1[:],
                                         bias_r[:, nv * NV:(nv + 1) * NV],
                                         start=True, stop=False,
                                         skip_group_check=True)
                        for k in range(KC):
                            lh = tiall[:, c, k * 256 + half * 128:
                                       k * 256 + half * 128 + 128]
                            nc.tensor.matmul(po[:], lh, wts[k][:],
                                             start=False, stop=(k == KC - 1),
                                             skip_group_check=True)
                        osb = osbp.tile([128, NV], F32, tag="osb")
                        nc.scalar.copy(osb[:], po[:])
                        nc.sync.dma_start(out_d[m * 128:(m + 1) * 128,
                                                nv * NV:(nv + 1) * NV], osb[:])

    nc.compile()
    return nc


_NC_CACHE = {}
_last_in_maps = None


def _get_nc():
    if "nc" not in _NC_CACHE:
        _NC_CACHE["nc"] = _build_nc()
    return _NC_CACHE["nc"]


def kernel(input_ids, emb_table, A, B, C, D, Wp, bp):
    input_ids = np.asarray(input_ids)
    emb_table = np.ascontiguousarray(np.asarray(emb_table), dtype=np.float32)
    A = np.asarray(A, dtype=np.float32)
    B = np.asarray(B, dtype=np.float32)
    C = np.asarray(C, dtype=np.float32)
    D = np.asarray(D, dtype=np.float32)
    Wp = np.asarray(Wp, dtype=np.float32)
    bp = np.asarray(bp, dtype=np.float32)

    ids_flat = input_ids.reshape(-1).astype(np.int32)          # (2048,)

    at = np.ascontiguousarray(A.transpose(0, 2, 1))            # (L,128,128)
    at_hi, at_lo = _hilo(at)
    bt = np.ascontiguousarray(
        B.transpose(2, 0, 1).reshape(KC, 128, L, DS).transpose(2, 1, 0, 3))
    # bt[l,p,k,m] = B[l, m, k*128+p]
    bt_hi, bt_lo = _hilo(bt)
    ct = np.ascontiguousarray(C.transpose(0, 2, 1).reshape(L, 128, KC, 128))
    # ct[l,p,mc,m] = C[l, mc*128+m, p]
    ct_hi, ct_lo = _hilo(ct)
    dc = np.ascontiguousarray(D.reshape(L, KC, 128).transpose(0, 2, 1))  # (L,128,KC)
    dc_hi, dc_lo = _hilo(dc)

    wpt = np.ascontiguousarray(Wp.T)                           # (512, 32000) f32
    wpt_bf = wpt.astype(ml_dtypes.bfloat16)

    nc = _get_nc()
    in_maps = []
    for c in range(NCORES):
        ids_c = ids_flat[c * TOKPC:(c + 1) * TOKPC].reshape(2, 128, 1)
        in_maps.append({
            "ids": np.ascontiguousarray(ids_c),
            "emb": emb_table,
            "at_hi": at_hi, "at_lo": at_lo,
            "bt_hi": bt_hi, "bt_lo": bt_lo,
            "ct_hi": ct_hi, "ct_lo": ct_lo,
            "dc_hi": dc_hi, "dc_lo": dc_lo,
            "wpt": np.ascontiguousarray(wpt_bf[:, c * VSH:(c + 1) * VSH]),
            "bias": np.ascontiguousarray(bp[c * VSH:(c + 1) * VSH]).reshape(1, VSH),
        })

    global _last_in_maps
    _last_in_maps = in_maps
    res = run_bass_kernel_spmd(nc, in_maps, core_ids=list(range(NCORES)))
    outs = [res.results[c]["out"] for c in range(NCORES)]
    full = np.concatenate(outs, axis=1)                        # (2048, 32000)
    return full.reshape(BATCH, SEQ, VOC).astype(np.float32)


# revision 25
# speedup vs baseline: 1.5234x; 1.0486x over previous
"""Trainium2 Bass kernel for nn_BreakthroughSNN (spiking SSM LM).

Strategy (8 NeuronCores, SPMD single NEFF):
  - Data-parallel SSM: 2048 tokens (B*S) sharded 256/core. Per core, the
    4-layer x 20-step LIF recurrence runs with persistent membrane
    potentials held in PSUM (PE accumulates state/output updates, DVE
    applies the leak*reset in place, ACT computes sign(v-thr), GPSIMD
    emits spikes).
  - All SSM matmuls are fp32r hi/lo pairs (host-split so the device's
    fp32r rounding is exact) -> full fp32-grade precision at 1 cyc/row.
  - Temporal encoding via host-precomputed exact fp32 sigmoid-boundary
    thresholds (no device sigmoid -> bit-exact one-hot vs fp32 ref).
  - Vocab-sharded output projection: time-integrated rates are
    AllGathered (bf16, tiny) so each core computes all 2048 tokens x its
    4000-vocab shard; Wp streamed as bf16 (post-chaos linear op).
"""

import numpy as np
import ml_dtypes
from contextlib import ExitStack

import concourse.bass as bass
import concourse.mybir as mybir
import concourse.tile as tile
from concourse import bacc
from concourse.bass_utils import run_bass_kernel_spmd
from concourse.masks import make_identity

F32 = mybir.dt.float32
F32R = mybir.dt.float32r
BF16 = mybir.dt.bfloat16
I32 = mybir.dt.int32
OP = mybir.AluOpType
ACTF = mybir.ActivationFunctionType

NCORES = 8
TOKPC = 256          # tokens per core
BATCH, SEQ = 4, 512
DM, DS = 512, 128
T, L = 20, 4
VOC = 32000
VSH = VOC // NCORES  # 4000 vocab per core
NV = 500             # vocab cols per proj tile (one PSUM bank; 8 tiles per core)
KC = DM // 128       # 4 feature chunks


def _hilo(x):
    x = np.ascontiguousarray(x, dtype=np.float32)
    u = x.view(np.uint32)
    hi = (u & np.uint32(0xFFFFF000)).view(np.float32).copy()  # keep 11 mantissa bits
    lo = (x - hi).astype(np.float32)
    return hi, lo


def _f2key(x):
    u = int(np.array(x, dtype=np.float32).view(np.uint32))
    return (u ^ 0x80000000) if u < 0x80000000 else (0xFFFFFFFF - u)


def _key2f(k):
    u = (k ^ 0x80000000) if k >= 0x80000000 else (0xFFFFFFFF - k)
    return np.array([u], dtype=np.uint32).view(np.float32)[0]


def _g32(x):
    # replicate reference fp32 pipeline: floor happens on this value
    x = np.float32(x)
    s = np.float32(1.0) / (np.float32(1.0) + np.float32(np.exp(np.float32(-x))))
    return np.float32(s * np.float32(19.0))


def _thresholds():
    """T_k = smallest fp32 x with g32(x) >= k, k=1..19 (g32 monotone)."""
    ts = []
    for k in range(1, 20):
        lo_k = _f2key(np.float32(-30.0))
        hi_k = _f2key(np.float32(30.0))
        assert _g32(_key2f(hi_k)) >= k and _g32(_key2f(lo_k)) < k
        while hi_k - lo_k > 1:
            mid = (lo_k + hi_k) // 2
            if _g32(_key2f(mid)) >= k:
                hi_k = mid
            else:
                lo_k = mid
        ts.append(float(_key2f(hi_k)))
    return ts


def _build_nc():
    nc = bacc.Bacc("TRN2", target_bir_lowering=False, debug=False, num_devices=NCORES)

    ids_d = nc.dram_tensor("ids", [2, 128, 1], I32, kind="ExternalInput")
    emb_d = nc.dram_tensor("emb", [VOC, DM], F32, kind="ExternalInput")
    at_hi_d = nc.dram_tensor("at_hi", [L, 128, 128], F32, kind="ExternalInput")
    at_lo_d = nc.dram_tensor("at_lo", [L, 128, 128], F32, kind="ExternalInput")
    bt_hi_d = nc.dram_tensor("bt_hi", [L, 128, KC, 128], F32, kind="ExternalInput")
    bt_lo_d = nc.dram_tensor("bt_lo", [L, 128, KC, 128], F32, kind="ExternalInput")
    ct_hi_d = nc.dram_tensor("ct_hi", [L, 128, KC, 128], F32, kind="ExternalInput")
    ct_lo_d = nc.dram_tensor("ct_lo", [L, 128, KC, 128], F32, kind="ExternalInput")
    dc_hi_d = nc.dram_tensor("dc_hi", [L, 128, KC], F32, kind="ExternalInput")
    dc_lo_d = nc.dram_tensor("dc_lo", [L, 128, KC], F32, kind="ExternalInput")
    wpt_d = nc.dram_tensor("wpt", [DM, VSH], BF16, kind="ExternalInput")
    bias_d = nc.dram_tensor("bias", [1, VSH], F32, kind="ExternalInput")
    out_d = nc.dram_tensor("out", [TOKPC * NCORES, VSH], F32, kind="ExternalOutput")

    THR = _thresholds()

    with tile.TileContext(nc) as tc, ExitStack() as ctx:
        const = ctx.enter_context(tc.tile_pool(name="const", bufs=1))
        ident = const.tile([128, 128], F32)
        make_identity(nc, ident[:])
        ident_r = const.tile([128, 128], F32R)
        nc.vector.tensor_copy(ident_r[:], ident[:])
        neg2 = const.tile([128, 1], F32)
        nc.vector.memset(neg2[:], -2.0)

        xb_pool = ctx.enter_context(tc.tile_pool(name="xb", bufs=1))
        xb = xb_pool.tile([128, T * KC * 256], F32R)
        tip = ctx.enter_context(tc.tile_pool(name="ti", bufs=1))
        tibf = tip.tile([128, KC * 256], BF16, tag="tibf")

        # ---------------- encode: gather + transpose + thresholds ----------
        with tc.tile_pool(name="enc", bufs=2) as enc, \
             tc.tile_pool(name="encp", bufs=2, space="PSUM") as encps, \
             tc.tile_pool(name="emb4", bufs=1) as emb4:
            ids_s = enc.tile([128, 2], I32, tag="ids")
            for g in range(2):
                nc.sync.dma_start(ids_s[:, g:g + 1], ids_d[g, :, :])
            EMB = [emb4.tile([128, TOKPC], F32, tag=f"emb{k}", name=f"EMB{k}") for k in range(KC)]
            IDX = [emb4.tile([128, TOKPC], F32, tag=f"idx{k}", name=f"IDX{k}") for k in range(KC)]
            for g in range(2):
                eg = enc.tile([128, DM], F32, tag="eg")
                nc.gpsimd.indirect_dma_start(
                    out=eg[:], out_offset=None,
                    in_=emb_d[:, :],
                    in_offset=bass.IndirectOffsetOnAxis(ap=ids_s[:, g:g + 1], axis=0),
                )
                for k in range(KC):
                    pt = encps.tile([128, 128], F32, tag="pt")
                    nc.tensor.transpose(pt[:], eg[:, k * 128:(k + 1) * 128], ident[:])
                    nc.scalar.copy(EMB[k][:, g * 128:(g + 1) * 128], pt[:])
            for k in range(KC):
                nc.vector.memset(IDX[k][:], 0.0)
                for j, tj in enumerate(THR):
                    nc.vector.scalar_tensor_tensor(IDX[k][:], EMB[k][:], float(tj),
                                                   IDX[k][:], OP.is_ge, OP.add)
            # one-hot spikes into X buffer (values {0,1}, fp32r-exact)
            for t in range(T):
                for k in range(KC):
                    nc.vector.tensor_scalar(
                        xb[:, (t * KC + k) * 256:(t * KC + k) * 256 + 256],
                        IDX[k][:], float(t), None, OP.is_equal)

        # ---------------- SSM layers ---------------------------------------
        with tc.tile_pool(name="ssmp", bufs=1, space="PSUM") as ssmps, \
             tc.tile_pool(name="par", bufs=2) as par, \
             tc.tile_pool(name="stg", bufs=2) as stg, \
             tc.tile_pool(name="lif", bufs=3) as lif:
            v1ps = ssmps.tile([128, TOKPC], F32, tag="v1")
            # v2 as two (128,512) tiles: pair j holds feature chunks 2j, 2j+1
            # side by side in the free dim (each tile = exactly one PSUM bank)
            v2pr = [ssmps.tile([128, 2 * TOKPC], F32, tag=f"v2p{j}", name=f"v2pr{j}")
                    for j in range(2)]
            tips = ssmps.tile([128, KC * TOKPC], F32, tag="tips")

            Hprev = None
            for layer in range(L):
                # -- param prep (hi/lo fp32r tiles; host pre-rounded) --
                def load_rounded(dram_ap, shape, tag):
                    st = stg.tile(list(shape), F32, tag="stage")
                    nc.sync.dma_start(st[:], dram_ap)
                    pt_ = par.tile(list(shape), F32R, tag=tag, name=f"par_{tag}")
                    nc.vector.tensor_copy(pt_[:], st[:])
                    return pt_

                ah = load_rounded(at_hi_d[layer, :, :], (128, 128), "ah")
                al = load_rounded(at_lo_d[layer, :, :], (128, 128), "al")
                bh = load_rounded(bt_hi_d[layer, :, :, :], (128, KC, 128), "bh")
                bl = load_rounded(bt_lo_d[layer, :, :, :], (128, KC, 128), "bl")
                ch = load_rounded(ct_hi_d[layer, :, :, :], (128, KC, 128), "ch")
                cl = load_rounded(ct_lo_d[layer, :, :, :], (128, KC, 128), "cl")
                dch = stg.tile([128, KC], F32, tag="dch")
                nc.sync.dma_start(dch[:], dc_hi_d[layer, :, :])
                dcl = stg.tile([128, KC], F32, tag="dcl")
                nc.sync.dma_start(dcl[:], dc_lo_d[layer, :, :])
                ddh, ddl = [], []
                for k in range(KC):
                    dt_ = par.tile([128, 128], F32R, tag=f"ddh{k}", name=f"ddh{k}")
                    nc.vector.tensor_scalar(dt_[:], ident[:], dch[:, k:k + 1], None, OP.mult)
                    ddh.append(dt_)
                    dt_ = par.tile([128, 128], F32R, tag=f"ddl{k}", name=f"ddl{k}")
                    nc.vector.tensor_scalar(dt_[:], ident[:], dcl[:, k:k + 1], None, OP.mult)
                    ddl.append(dt_)

                def emit_mm2_lif2(t, H_t, xs_t, layer_):
                    # output update accumulation (v2, per chunk) + LIF2
                    for k in range(KC):
                        vsl = v2pr[k // 2][:, (k % 2) * TOKPC:(k % 2 + 1) * TOKPC]
                        mm2 = [(ch[:, k, :], H_t[:]), (cl[:, k, :], H_t[:]),
                               (ddh[k][:], xs_t[k]), (ddl[k][:], xs_t[k])]
                        for i, (lhsT, rhs) in enumerate(mm2):
                            # start=True clears the WHOLE bank -> only the
                            # first MM into each bank per layer may set it;
                            # the pair sibling begins on has_written=0.
                            nc.tensor.matmul(vsl, lhsT, rhs,
                                             start=(t == 0 and i == 0 and k % 2 == 0),
                                             stop=(i == len(mm2) - 1),
                                             skip_group_check=True)
                    for j in range(2):
                        xsl = xb[:, (t * KC + 2 * j) * 256:(t * KC + 2 * j) * 256 + 512]
                        m2 = lif.tile([128, 2 * TOKPC], F32, tag=f"m2_{j}",
                                      name=f"m2_{j}")
                        if j == 0:
                            # DVE straight from PSUM; spike via GPSIMD from m2
                            nc.vector.tensor_scalar(m2[:], v2pr[j][:], 2.0, 0.5,
                                                    OP.is_lt, OP.mult)
                            nc.gpsimd.tensor_scalar(xsl, m2[:], -2.0, 1.0,
                                                    OP.mult, OP.add)
                        else:
                            sg2 = lif.tile([128, 2 * TOKPC], F32, tag="sg2",
                                           name="sg2")
                            nc.scalar.activation(sg2[:], v2pr[j][:], ACTF.Sign,
                                                 bias=neg2[:], scale=1.0)
                            nc.gpsimd.tensor_scalar(m2[:], sg2[:], -0.25, 0.25,
                                                    OP.mult, OP.add)
                            nc.gpsimd.tensor_scalar(xsl, sg2[:], 0.5, 0.5,
                                                    OP.mult, OP.add)
                        nc.vector.tensor_tensor(v2pr[j][:], v2pr[j][:], m2[:], OP.mult)
                    if layer_ == L - 1:
                        # time-integration on the PE: tips += I @ X[t]
                        # (tips spans 2 banks: slices 0,1 / 2,3 -> one
                        # start=True per bank, at k==0 and k==2)
                        for k in range(KC):
                            nc.tensor.matmul(
                                tips[:, k * TOKPC:(k + 1) * TOKPC],
                                ident_r[:], xs_t[k],
                                start=(t == 0 and k % 2 == 0),
                                stop=(t == T - 1),
                                skip_group_check=True)

                prev = None  # (t, H, xs) pending MM2+LIF2 (1-step software skew)
                for t in range(T):
                    xs = [xb[:, (t * KC + k) * 256:(t * KC + k) * 256 + 256]
                          for k in range(KC)]
                    # ---- state update accumulation (v1) ----
                    mm1 = []
                    if t > 0:
                        mm1 += [(ah[:], Hprev[:]), (al[:], Hprev[:])]
                    for k in range(KC):
                        mm1 += [(bh[:, k, :], xs[k]), (bl[:, k, :], xs[k])]
                    for i, (lhsT, rhs) in enumerate(mm1):
                        nc.tensor.matmul(v1ps[:], lhsT, rhs,
                                         start=(t == 0 and i == 0),
                                         stop=(i == len(mm1) - 1),
                                         skip_group_check=True)
                    # ---- LIF1 (m1 straight from PSUM; spike via GPSIMD) ----
                    m1 = lif.tile([128, TOKPC], F32, tag="m1")
                    nc.vector.tensor_scalar(m1[:], v1ps[:], 2.0, 0.5, OP.is_lt, OP.mult)
                    H = lif.tile([128, TOKPC], F32R, tag="H", bufs=3)
                    nc.gpsimd.tensor_scalar(H[:], m1[:], -2.0, 1.0, OP.mult, OP.add)
                    nc.vector.tensor_tensor(v1ps[:], v1ps[:], m1[:], OP.mult)
                    # ---- previous step's output-side work (keeps PE fed) ----
                    if prev is not None:
                        emit_mm2_lif2(*prev, layer)
                    prev = (t, H, xs)
                    Hprev = H
                emit_mm2_lif2(*prev, layer)

            # time-integrated rates -> bf16 (tips psum holds sum over T)
            for k in range(KC):
                nc.vector.tensor_scalar(tibf[:, k * 256:(k + 1) * 256],
                                        tips[:, k * TOKPC:(k + 1) * TOKPC],
                                        1.0 / T, None, OP.mult)

        # ---------------- allgather + projection ----------------------------
        with tc.tile_pool(name="agd", bufs=1, space="DRAM") as agd:
            agi = agd.tile([128, KC * 256], BF16)
            nc.sync.dma_start(agi[:], tibf[:])
            ago = agd.tile([NCORES * 128, KC * 256], BF16)
            nc.gpsimd.collective_compute(
                "AllGather", OP.bypass,
                replica_groups=[list(range(NCORES))],
                ins=[agi[:].opt()], outs=[ago[:].opt()],
            )
            tiall = tip.tile([128, NCORES, KC * 256], BF16, tag="tiall")
            nc.sync.dma_start(
                tiall[:],
                ago[:].rearrange("(n p) x -> p n x", n=NCORES, p=128))

            # ---------------- vocab-sharded projection ---------------------
            with tc.tile_pool(name="prj", bufs=2) as prj, \
                 tc.tile_pool(name="prjp", bufs=4, space="PSUM") as prjps, \
                 tc.tile_pool(name="osb", bufs=4) as osbp:
                mchunks = TOKPC * NCORES // 128
                for nv in range(VSH // NV):
                    bias_bc = prj.tile([128, NV], F32, tag="bias")
                    bap = bias_d[0:1, nv * NV:(nv + 1) * NV]
                    bsrc = bass.AP(tensor=bap.tensor, offset=bap.offset,
                                   ap=[[0, 128], [1, NV]])
                    nc.sync.dma_start(bias_bc[:], bsrc)
                    wts = []
                    for k in range(KC):
                        wt = prj.tile([128, NV], BF16, tag=f"wt{k}", name=f"wt{k}")
                        nc.sync.dma_start(wt[:], wpt_d[k * 128:(k + 1) * 128,
                                                       nv * NV:(nv + 1) * NV])
                        wts.append(wt)
                    for m in range(mchunks):
                        c, half = divmod(m, 2)
                        po = prjps.tile([128, NV], F32, tag="po")
                        for k in range(KC):
                            lh = tiall[:, c, k * 256 + half * 128:
                                       k * 256 + half * 128 + 128]
                            nc.tensor.matmul(po[:], lh, wts[k][:],
                                             start=(k == 0), stop=(k == KC - 1),
                                             skip_group_check=True)
                        osb = osbp.tile([128, NV], F32, tag="osb")
                        if m % 2 == 0:
                            nc.vector.tensor_tensor(osb[:], po[:], bias_bc[:], OP.add)
                        else:
                            nc.scalar.copy(osb[:], po[:])
                            nc.gpsimd.tensor_tensor(osb[:], osb[:], bias_bc[:], OP.add)
                        nc.sync.dma_start(out_d[m * 128:(m + 1) * 128,
                                                nv * NV:(nv + 1) * NV], osb[:])

    nc.compile()
    return nc


_NC_CACHE = {}
_last_in_maps = None


def _get_nc():
    if "nc" not in _NC_CACHE:
        _NC_CACHE["nc"] = _build_nc()
    return _NC_CACHE["nc"]


def kernel(input_ids, emb_table, A, B, C, D, Wp, bp):
    input_ids = np.asarray(input_ids)
    emb_table = np.ascontiguousarray(np.asarray(emb_table), dtype=np.float32)
    A = np.asarray(A, dtype=np.float32)
    B = np.asarray(B, dtype=np.float32)
    C = np.asarray(C, dtype=np.float32)
    D = np.asarray(D, dtype=np.float32)
    Wp = np.asarray(Wp, dtype=np.float32)
    bp = np.asarray(bp, dtype=np.float32)

    ids_flat = input_ids.reshape(-1).astype(np.int32)          # (2048,)

    at = np.ascontiguousarray(A.transpose(0, 2, 1))            # (L,128,128)
    at_hi, at_lo = _hilo(at)
    bt = np.ascontiguousarray(
        B.transpose(2, 0, 1).reshape(KC, 128, L, DS).transpose(2, 1, 0, 3))
    # bt[l,p,k,m] = B[l, m, k*128+p]
    bt_hi, bt_lo = _hilo(bt)
    ct = np.ascontiguousarray(C.transpose(0, 2, 1).reshape(L, 128, KC, 128))
    # ct[l,p,mc,m] = C[l, mc*128+m, p]
    ct_hi, ct_lo = _hilo(ct)
    dc = np.ascontiguousarray(D.reshape(L, KC, 128).transpose(0, 2, 1))  # (L,128,KC)
    dc_hi, dc_lo = _hilo(dc)

    wpt = np.ascontiguousarray(Wp.T)                           # (512, 32000) f32
    wpt_bf = wpt.astype(ml_dtypes.bfloat16)

    nc = _get_nc()
    in_maps = []
    for c in range(NCORES):
        ids_c = ids_flat[c * TOKPC:(c + 1) * TOKPC].reshape(2, 128, 1)
        in_maps.append({
            "ids": np.ascontiguousarray(ids_c),
            "emb": emb_table,
            "at_hi": at_hi, "at_lo": at_lo,
            "bt_hi": bt_hi, "bt_lo": bt_lo,
            "ct_hi": ct_hi, "ct_lo": ct_lo,
            "dc_hi": dc_hi, "dc_lo": dc_lo,
            "wpt": np.ascontiguousarray(wpt_bf[:, c * VSH:(c + 1) * VSH]),
            "bias": np.ascontiguousarray(bp[c * VSH:(c + 1) * VSH]).reshape(1, VSH),
        })

    global _last_in_maps
    _last_in_maps = in_maps
    res = run_bass_kernel_spmd(nc, in_maps, core_ids=list(range(NCORES)))
    outs = [res.results[c]["out"] for c in range(NCORES)]
    full = np.concatenate(outs, axis=1)                        # (2048, 32000)
    return full.reshape(BATCH, SEQ, VOC).astype(np.float32)
